# revision 2
# baseline (speedup 1.0000x reference)
"""MultiHeadAttentionBlock (cosine attention) Bass kernel for 8 Trainium2 cores.

Reference computation (B=4, S=2048, D=512, H=8, Dk=64):
  Q = LN(q@Wq.T); K = LN(k@Wk.T); V = v@Wv.T
  Q,K L2-normalized over D; per-head scores = Q K^T / 8, mask-filled, softmax
  out = (softmax @ V) reshaped @ Wo.T

Sharding: core c handles batch b=c//2, query-row half h=c%2 (1024 rows).
K/V/LN work for a batch is duplicated across its 2 cores (no collectives).
"""

import numpy as np

B, S, D, H, DK = 4, 2048, 512, 8, 64
TOKQ = S // 2          # query rows per core
NQT = TOKQ // 128      # 8 query token tiles
NKT = S // 128         # 16 key/value token tiles
LN_EPS = 1e-5
L2_EPS = 1e-6
SQRT_D = float(np.sqrt(D))

_cache = {}


def _build(apply_gamma_beta: bool):
    import concourse.mybir as mybir
    import concourse.tile as tile
    from concourse import bacc
    from concourse.masks import make_identity

    f32 = mybir.dt.float32
    f32r = mybir.dt.float32r
    i32 = mybir.dt.int32
    Alu = mybir.AluOpType
    Act = mybir.ActivationFunctionType

    nc = bacc.Bacc("TRN2", target_bir_lowering=False, debug=False)

    q_d = nc.dram_tensor("q", [TOKQ, D], f32, kind="ExternalInput")
    k_d = nc.dram_tensor("k", [S, D], f32, kind="ExternalInput")
    v_d = nc.dram_tensor("v", [S, D], f32, kind="ExternalInput")
    m_d = nc.dram_tensor("mask", [S], i32, kind="ExternalInput")
    w_d = {n: nc.dram_tensor(n, [D, D], f32, kind="ExternalInput")
           for n in ("wq", "wk", "wv", "wo")}
    if apply_gamma_beta:
        gb_d = {n: nc.dram_tensor(n, [D], f32, kind="ExternalInput")
                for n in ("gq", "bq", "gk", "bk")}
    out_d = nc.dram_tensor("out", [TOKQ, D], f32, kind="ExternalOutput")

    with tile.TileContext(nc) as tc:
        with (
            tc.tile_pool(name="persist", bufs=1) as persist,
            tc.tile_pool(name="xin", bufs=3) as xin,
            tc.tile_pool(name="xt", bufs=3) as xt,
            tc.tile_pool(name="norm", bufs=3) as norm,
            tc.tile_pool(name="stats", bufs=4) as stats_pool,
            tc.tile_pool(name="probs", bufs=3) as probs_pool,
            tc.tile_pool(name="fin", bufs=2) as fin_pool,
        ):
            ident = persist.tile([128, 128], f32, tag="ident")
            make_identity(nc, ident)
            eps_ln = persist.tile([128, 1], f32, tag="eps_ln")
            nc.vector.memset(eps_ln, LN_EPS)
            ones1 = persist.tile([1, 64], f32, tag="ones1")
            nc.vector.memset(ones1, 1.0)
            ones64 = persist.tile([1, 64], f32r, tag="ones64")
            nc.scalar.copy(out=ones64, in_=ones1)

            # ---- weights: transpose each [512,512] W into WT[p=in_chunk, ci, out]
            wT = {}
            with (
                tc.tile_pool(name="ps_t", bufs=2, space="PSUM") as ps_t,
                tc.tile_pool(name="ps_p", bufs=2, space="PSUM") as ps_p,
            ):
                for name in ("wq", "wk", "wv", "wo"):
                    w_sb = xin.tile([128, 4, D], f32, tag="w_in")
                    nc.sync.dma_start(
                        out=w_sb, in_=w_d[name].rearrange("(c p) f -> p c f", p=128))
                    wt_sb = persist.tile([128, 4, D], f32r, tag=f"wT_{name}")
                    for ci in range(4):
                        wt_ps = ps_t.tile([128, D], f32, tag="tr")
                        for co in range(4):
                            nc.tensor.transpose(
                                out=wt_ps[:, co * 128:(co + 1) * 128],
                                in_=w_sb[:, co, ci * 128:(ci + 1) * 128],
                                identity=ident)
                        nc.scalar.copy(out=wt_sb[:, ci, :], in_=wt_ps)
                    wT[name] = wt_sb

                # ---- mask -> per-key exp bias [128, 16] f32 (0 or -10000)
                msk_i = persist.tile([16, 128], i32, tag="msk_i")
                nc.sync.dma_start(out=msk_i, in_=m_d.rearrange("(c p) -> c p", p=128))
                msk_f = persist.tile([16, 128], f32, tag="msk_f")
                nc.vector.tensor_copy(out=msk_f, in_=msk_i)
                msk_ps = ps_t.tile([128, 16], f32, tag="msk_ps")
                nc.tensor.transpose(out=msk_ps, in_=msk_f, identity=ident[0:16, 0:16])
                mask_bias = persist.tile([128, 16], f32, tag="mask_bias")
                nc.scalar.activation(out=mask_bias, in_=msk_ps, func=Act.Copy,
                                     bias=-10000.0, scale=10000.0)

                if apply_gamma_beta:
                    gb_sb = {}
                    for n in ("gq", "bq", "gk", "bk"):
                        t = persist.tile([128, D], f32, tag=f"gb_{n}")
                        import concourse.bass as bass_mod
                        src = gb_d[n][:]
                        bcast = bass_mod.AP(
                            tensor=src.tensor, offset=src.offset,
                            ap=[[0, 128]] + list(src.ap))
                        nc.sync.dma_start(out=t, in_=bcast)
                        gb_sb[n] = t

                # ---- persistent activation stores
                qT = persist.tile([128, 4, TOKQ], f32r, tag="qT")     # Q_norm^T
                kT = persist.tile([128, 4, S], f32r, tag="kT")        # K_norm^T
                vS = persist.tile([128, NKT, H, DK + 1], f32r, tag="vS")
                oT = persist.tile([128, 4, TOKQ], f32r, tag="oT")     # O_hat^T

                ones_col = persist.tile([128, NKT, H, 1], f32, tag="ones_col")
                nc.vector.memset(ones_col, 1.0)
                nc.scalar.copy(out=vS[:, :, :, DK:DK + 1], in_=ones_col)

                # ---- projection + LN + L2 for Q and K; projection only for V
                def proj_tile(src_dram, row0, wt_sb):
                    """DMA a [128, D] token tile, transpose, project. Returns psum."""
                    x_sb = xin.tile([128, D], f32, tag="x_in")
                    nc.sync.dma_start(out=x_sb, in_=src_dram[row0:row0 + 128, :])
                    xT_ps = ps_t.tile([128, 4, 128], f32, tag="tr")
                    for ci in range(4):
                        nc.tensor.transpose(
                            out=xT_ps[:, ci, :],
                            in_=x_sb[:, ci * 128:(ci + 1) * 128], identity=ident)
                    xT_sb = xt.tile([128, 4, 128], f32r, tag="xT")
                    nc.scalar.copy(out=xT_sb, in_=xT_ps)
                    p_ps = ps_p.tile([128, D], f32, tag="proj")
                    for ci in range(4):
                        nc.tensor.matmul(out=p_ps, lhsT=xT_sb[:, ci, :],
                                         rhs=wt_sb[:, ci, :],
                                         start=(ci == 0), stop=(ci == 3))
                    return p_ps

                def ln_l2(p_ps, g_name, b_name):
                    """LayerNorm + L2-normalize rows of p_ps. Returns [128, D] f32 SBUF."""
                    st = stats_pool.tile([128, 6], f32, tag="bn")
                    nc.vector.bn_stats(out=st, in_=p_ps)
                    mv = stats_pool.tile([128, 2], f32, tag="mv")
                    nc.vector.bn_aggr(out=mv, in_=st)
                    mean, var = mv[:, 0:1], mv[:, 1:2]
                    std = stats_pool.tile([128, 1], f32, tag="std")
                    nc.scalar.activation(out=std, in_=var, func=Act.Sqrt, bias=eps_ln)
                    rstd = stats_pool.tile([128, 1], f32, tag="rstd")
                    nc.vector.reciprocal(out=rstd, in_=std)
                    y_sb = norm.tile([128, D], f32, tag="y")
                    if not apply_gamma_beta:
                        # ||(x-mean)*rstd|| = sqrt(D*var)*rstd, computable from stats
                        sqv = stats_pool.tile([128, 1], f32, tag="sqv")
                        nc.scalar.activation(out=sqv, in_=var, func=Act.Sqrt, bias=0.0)
                        t0 = stats_pool.tile([128, 1], f32, tag="t0")
                        nc.vector.tensor_tensor(out=t0, in0=sqv, in1=rstd, op=Alu.mult)
                        dn = stats_pool.tile([128, 1], f32, tag="dn")
                        nc.scalar.activation(out=dn, in_=t0, func=Act.Copy,
                                             bias=L2_EPS, scale=SQRT_D)
                        l2r = stats_pool.tile([128, 1], f32, tag="l2r")
                        nc.vector.reciprocal(out=l2r, in_=dn)
                        sc = stats_pool.tile([128, 1], f32, tag="sc")
                        nc.vector.tensor_tensor(out=sc, in0=rstd, in1=l2r, op=Alu.mult)
                        nc.vector.tensor_scalar(out=y_sb, in0=p_ps, scalar1=mean,
                                                scalar2=sc, op0=Alu.subtract,
                                                op1=Alu.mult)
                    else:
                        nc.vector.tensor_scalar(out=y_sb, in0=p_ps, scalar1=mean,
                                                scalar2=rstd, op0=Alu.subtract,
                                                op1=Alu.mult)
                        nc.vector.tensor_tensor(out=y_sb, in0=y_sb,
                                                in1=gb_sb[g_name], op=Alu.mult)
                        nc.vector.tensor_tensor(out=y_sb, in0=y_sb,
                                                in1=gb_sb[b_name], op=Alu.add)
                        scr = norm.tile([128, D], f32, tag="scr")
                        ssq = stats_pool.tile([128, 1], f32, tag="ssq")
                        nc.scalar.activation(out=scr, in_=y_sb, func=Act.Square,
                                             accum_out=ssq)
                        dn = stats_pool.tile([128, 1], f32, tag="dn")
                        nc.scalar.activation(out=dn, in_=ssq, func=Act.Sqrt, bias=0.0)
                        dn2 = stats_pool.tile([128, 1], f32, tag="dn2")
                        nc.scalar.activation(out=dn2, in_=dn, func=Act.Copy,
                                             bias=L2_EPS, scale=1.0)
                        l2r = stats_pool.tile([128, 1], f32, tag="l2r")
                        nc.vector.reciprocal(out=l2r, in_=dn2)
                        nc.vector.tensor_scalar_mul(out=y_sb, in0=y_sb, scalar1=l2r)
                    return y_sb

                def store_T(y_sb, dst, col0):
                    """Transpose [128, D] -> dst[:, :, col0:col0+128] (f32r)."""
                    t_ps = ps_t.tile([128, 4, 128], f32, tag="tr")
                    for ci in range(4):
                        nc.tensor.transpose(
                            out=t_ps[:, ci, :],
                            in_=y_sb[:, ci * 128:(ci + 1) * 128], identity=ident)
                    nc.scalar.copy(out=dst[:, :, col0:col0 + 128], in_=t_ps)

                for t in range(NQT):
                    p_ps = proj_tile(q_d, t * 128, wT["wq"])
                    y_sb = ln_l2(p_ps, "gq", "bq")
                    store_T(y_sb, qT, t * 128)

                for t in range(NKT):
                    p_ps = proj_tile(k_d, t * 128, wT["wk"])
                    y_sb = ln_l2(p_ps, "gk", "bk")
                    store_T(y_sb, kT, t * 128)

                for t in range(NKT):
                    p_ps = proj_tile(v_d, t * 128, wT["wv"])
                    nc.scalar.copy(
                        out=vS[:, t, :, 0:DK],
                        in_=p_ps.rearrange("p (h d) -> p h d", h=H))

            # ---- attention + output projection
            with (
                tc.tile_pool(name="ps_s", bufs=2, space="PSUM") as ps_s,
                tc.tile_pool(name="ps_o", bufs=2, space="PSUM") as ps_o,
                tc.tile_pool(name="ps_b", bufs=2, space="PSUM") as ps_b,
                tc.tile_pool(name="ps_f", bufs=2, space="PSUM") as ps_f,
            ):
                for qt in range(TOKQ // 512):
                    for h in range(8):
                        po = h % 2
                        ch = h // 2
                        o_ps = ps_o.tile([DK + 1, 512], f32, tag="o")
                        for kc in range(NKT):
                            s_ps = ps_s.tile([128, 512], f32, tag="s")
                            nc.tensor.matmul(
                                out=s_ps,
                                lhsT=kT[po * 64:(po + 1) * 64, ch,
                                        kc * 128:(kc + 1) * 128],
                                rhs=qT[po * 64:(po + 1) * 64, ch,
                                       qt * 512:(qt + 1) * 512],
                                start=True, stop=True)
                            pr_sb = probs_pool.tile([128, 512], f32r, tag="pr")
                            nc.scalar.activation(
                                out=pr_sb, in_=s_ps, func=Act.Exp,
                                bias=mask_bias[:, kc:kc + 1], scale=1.0 / 8.0)
                            nc.tensor.matmul(
                                out=o_ps, lhsT=vS[:, kc, h, :], rhs=pr_sb,
                                start=(kc == 0), stop=(kc == NKT - 1))
                        rec_sb = stats_pool.tile([1, 512], f32, tag="rec")
                        nc.vector.reciprocal(out=rec_sb, in_=o_ps[DK:DK + 1, :])
                        recr_sb = stats_pool.tile([1, 512], f32r, tag="recr")
                        nc.scalar.copy(out=recr_sb, in_=rec_sb)
                        b_ps = ps_b.tile([64, 512], f32, tag="b")
                        nc.tensor.matmul(out=b_ps, lhsT=ones64, rhs=recr_sb,
                                         start=True, stop=True)
                        b_sb = probs_pool.tile([64, 512], f32, tag="b_sb")
                        nc.scalar.copy(out=b_sb, in_=b_ps)
                        nc.vector.tensor_tensor(
                            out=oT[po * 64:(po + 1) * 64, ch,
                                   qt * 512:(qt + 1) * 512],
                            in0=o_ps[0:DK, :], in1=b_sb, op=Alu.mult)

                    for ts in range(4):
                        col0 = qt * 512 + ts * 128
                        f_ps = ps_f.tile([128, D], f32, tag="f")
                        for ci in range(4):
                            nc.tensor.matmul(out=f_ps,
                                             lhsT=oT[:, ci, col0:col0 + 128],
                                             rhs=wT["wo"][:, ci, :],
                                             start=(ci == 0), stop=(ci == 3))
                        f_sb = fin_pool.tile([128, D], f32, tag="f_sb")
                        nc.vector.tensor_copy(out=f_sb, in_=f_ps)
                        nc.sync.dma_start(out=out_d[col0:col0 + 128, :], in_=f_sb)

    nc.compile()
    return nc


def _get_nc(apply_gamma_beta):
    key = bool(apply_gamma_beta)
    if key not in _cache:
        _cache[key] = _build(key)
    return _cache[key]


def kernel(q, k, v, mask, Wq, Wk, Wv, Wo, gq, bq, gk, bk):
    from concourse.bass_utils import run_bass_kernel_spmd

    q = np.asarray(q, dtype=np.float32)
    k = np.asarray(k, dtype=np.float32)
    v = np.asarray(v, dtype=np.float32)
    mask = np.asarray(mask, dtype=np.int32)
    Wq, Wk = np.asarray(Wq, np.float32), np.asarray(Wk, np.float32)
    Wv, Wo = np.asarray(Wv, np.float32), np.asarray(Wo, np.float32)
    gq, bq = np.asarray(gq, np.float32), np.asarray(bq, np.float32)
    gk, bk = np.asarray(gk, np.float32), np.asarray(bk, np.float32)

    gb = not (np.all(gq == 1.0) and np.all(bq == 0.0)
              and np.all(gk == 1.0) and np.all(bk == 0.0))
    nc = _get_nc(gb)

    in_maps = []
    for c in range(8):
        b_, h_ = c // 2, c % 2
        m = {
            "q": np.ascontiguousarray(q[b_, h_ * TOKQ:(h_ + 1) * TOKQ]),
            "k": np.ascontiguousarray(k[b_]),
            "v": np.ascontiguousarray(v[b_]),
            "mask": np.ascontiguousarray(mask[b_, 0, 0]),
            "wq": Wq, "wk": Wk, "wv": Wv, "wo": Wo,
        }
        if gb:
            m.update({"gq": gq, "bq": bq, "gk": gk, "bk": bk})
        in_maps.append(m)

    res = run_bass_kernel_spmd(nc, in_maps, core_ids=list(range(8)))
    out = np.empty((B, S, D), np.float32)
    for c in range(8):
        b_, h_ = c // 2, c % 2
        out[b_, h_ * TOKQ:(h_ + 1) * TOKQ] = res.results[c]["out"]
    return out


# revision 10
# speedup vs baseline: 27.3101x; 27.3101x over previous
"""MultiHeadAttentionBlock (cosine attention) Bass kernel for 8 Trainium2 cores.

Reference computation (B=4, S=2048, D=512, H=8, Dk=64):
  Q = LN(q@Wq.T); K = LN(k@Wk.T); V = v@Wv.T
  Q,K L2-normalized over D; per-head scores = Q K^T / 8, mask-filled, softmax
  out = (softmax @ V) reshaped @ Wo.T

Sharding: core c handles batch b=c//2, query-row half h=c%2 (1024 rows).
K/V/LN work for a batch is duplicated across its 2 cores (no collectives).

Masked keys contribute exactly 0 to the softmax (exp(-10000) underflows in
f32, matching the reference bit-for-bit), so the host compacts K/V/mask to
just the unmasked rows (padded to a multiple of 128); the kernel is built
for that key count. With ~50% masking this halves the attention work.
"""

import numpy as np

B, S, D, H, DK = 4, 2048, 512, 8, 64
TOKQ = S // 2          # query rows per core
NQT = TOKQ // 128      # 8 query token tiles
LN_EPS = 1e-5
L2_EPS = 1e-6
SQRT_D = float(np.sqrt(D))

_cache = {}


def _build(apply_gamma_beta: bool, n_kc: int, repeat: int = 1):
    """n_kc: number of 128-row key chunks (compacted+padded key count / 128)."""
    import contextlib
    import concourse.bass as bass_mod
    import concourse.mybir as mybir
    import concourse.tile as tile
    from concourse import bacc
    from concourse.masks import make_identity

    f32 = mybir.dt.float32
    f32r = mybir.dt.float32r
    i32 = mybir.dt.int32
    Alu = mybir.AluOpType
    Act = mybir.ActivationFunctionType

    SK = n_kc * 128

    nc = bacc.Bacc("TRN2", target_bir_lowering=False, debug=False)

    q_d = nc.dram_tensor("q", [TOKQ, D], f32, kind="ExternalInput")
    k_d = nc.dram_tensor("k", [SK, D], f32, kind="ExternalInput")
    v_d = nc.dram_tensor("v", [SK, D], f32, kind="ExternalInput")
    m_d = nc.dram_tensor("mask", [SK], i32, kind="ExternalInput")
    w_d = {n: nc.dram_tensor(n, [D, D], f32, kind="ExternalInput")
           for n in ("wq", "wk", "wv", "wo")}
    if apply_gamma_beta:
        gb_d = {n: nc.dram_tensor(n, [D], f32, kind="ExternalInput")
                for n in ("gq", "bq", "gk", "bk")}
    out_d = nc.dram_tensor("out", [TOKQ, D], f32, kind="ExternalOutput")

    with tile.TileContext(nc) as tc:
        with (
            tc.tile_pool(name="persist", bufs=1) as persist,
            tc.tile_pool(name="xin", bufs=4) as xin,
            tc.tile_pool(name="xt", bufs=4) as xt,
            tc.tile_pool(name="norm", bufs=4) as norm,
            tc.tile_pool(name="stats", bufs=6) as stats_pool,
            tc.tile_pool(name="probs", bufs=4) as probs_pool,
            tc.tile_pool(name="fin", bufs=3) as fin_pool,
            tc.tile_pool(name="ps_a", bufs=2, space="PSUM") as ps_a,
            tc.tile_pool(name="ps_pf", bufs=3, space="PSUM") as ps_pf,
            tc.tile_pool(name="ps_o", bufs=2, space="PSUM") as ps_o,
            tc.tile_pool(name="ps_b", bufs=1, space="PSUM") as ps_b,
        ):
            loop_cm = tc.For_i(0, repeat, 1) if repeat > 1 else contextlib.nullcontext()
            with loop_cm:
                ident = persist.tile([128, 128], f32, tag="ident")
                make_identity(nc, ident)
                eps_ln = persist.tile([128, 1], f32, tag="eps_ln")
                nc.vector.memset(eps_ln, LN_EPS)
                eps_b2 = persist.tile([128, 1], f32, tag="eps_b2")
                nc.vector.memset(eps_b2, LN_EPS * L2_EPS * L2_EPS)
                zero_b = persist.tile([128, 1], f32, tag="zero_b")
                nc.vector.memset(zero_b, 0.0)
                ones1 = persist.tile([1, 64], f32, tag="ones1")
                nc.vector.memset(ones1, 1.0)
                ones64 = persist.tile([1, 64], f32r, tag="ones64")
                nc.scalar.copy(out=ones64, in_=ones1)

                # ---- weights: transpose [512,512] W into WT[p=in_chunk, ci, out]
                wT = {}
                for name in ("wq", "wk", "wv", "wo"):
                    w_sb = xin.tile([128, 4, D], f32, tag="w_in")
                    nc.sync.dma_start(
                        out=w_sb, in_=w_d[name].rearrange("(c p) f -> p c f", p=128))
                    wt_sb = persist.tile([128, 4, D], f32r, tag=f"wT_{name}")
                    for ci in range(4):
                        wt_ps = ps_a.tile([128, D], f32, tag="ts")
                        for co in range(4):
                            nc.tensor.transpose(
                                out=wt_ps[:, co * 128:(co + 1) * 128],
                                in_=w_sb[:, co, ci * 128:(ci + 1) * 128],
                                identity=ident)
                        nc.vector.tensor_copy(out=wt_sb[:, ci, :], in_=wt_ps)
                    wT[name] = wt_sb

                # ---- mask -> per-key exp bias [128, n_kc] f32 (0 or -10000)
                msk_i = persist.tile([n_kc, 128], i32, tag="msk_i")
                nc.sync.dma_start(out=msk_i, in_=m_d.rearrange("(c p) -> c p", p=128))
                msk_f = persist.tile([n_kc, 128], f32, tag="msk_f")
                nc.vector.tensor_copy(out=msk_f, in_=msk_i)
                msk_ps = ps_a.tile([128, n_kc], f32, tag="ts")
                nc.tensor.transpose(out=msk_ps, in_=msk_f,
                                    identity=ident[0:n_kc, 0:n_kc])
                mask_bias = persist.tile([128, n_kc], f32, tag="mask_bias")
                nc.scalar.activation(out=mask_bias, in_=msk_ps, func=Act.Copy,
                                     bias=-10000.0, scale=10000.0)

                if apply_gamma_beta:
                    gb_sb = {}
                    for n in ("gq", "bq", "gk", "bk"):
                        t = persist.tile([128, D], f32, tag=f"gb_{n}")
                        src = gb_d[n][:]
                        bcast = bass_mod.AP(
                            tensor=src.tensor, offset=src.offset,
                            ap=[[0, 128]] + list(src.ap))
                        nc.sync.dma_start(out=t, in_=bcast)
                        gb_sb[n] = t

                # ---- persistent activation stores
                qT = persist.tile([128, 4, TOKQ], f32r, tag="qT")     # Q_norm^T
                kT = persist.tile([128, 4, SK], f32r, tag="kT")       # K_norm^T
                vS = persist.tile([128, n_kc, H, DK + 1], f32r, tag="vS")
                oT = persist.tile([128, 4, TOKQ], f32r, tag="oT")     # O_hat^T

                ones_col = persist.tile([128, n_kc, H, 1], f32, tag="ones_col")
                nc.vector.memset(ones_col, 1.0)
                nc.scalar.copy(out=vS[:, :, :, DK:DK + 1], in_=ones_col)

                def proj_tile(src_dram, row0, wt_sb):
                    """DMA a [128, D] token tile, transpose, project. Returns psum."""
                    x_sb = xin.tile([128, D], f32, tag="x_in")
                    nc.sync.dma_start(out=x_sb, in_=src_dram[row0:row0 + 128, :])
                    xT_ps = ps_a.tile([128, 4, 128], f32, tag="ts")
                    for ci in range(4):
                        nc.tensor.transpose(
                            out=xT_ps[:, ci, :],
                            in_=x_sb[:, ci * 128:(ci + 1) * 128], identity=ident)
                    xT_sb = xt.tile([128, 4, 128], f32r, tag="xT")
                    nc.vector.tensor_copy(out=xT_sb, in_=xT_ps)
                    p_ps = ps_pf.tile([128, D], f32, tag="pf")
                    for ci in range(4):
                        nc.tensor.matmul(out=p_ps, lhsT=xT_sb[:, ci, :],
                                         rhs=wt_sb[:, ci, :],
                                         start=(ci == 0), stop=(ci == 3))
                    return p_ps

                def ln_l2(p_ps, g_name, b_name):
                    """LayerNorm + L2-normalize rows of p_ps. Returns [128, D] f32."""
                    st = stats_pool.tile([128, 6], f32, tag="bn")
                    nc.vector.bn_stats(out=st, in_=p_ps)
                    mv = stats_pool.tile([128, 2], f32, tag="mv")
                    nc.vector.bn_aggr(out=mv, in_=st)
                    mean, var = mv[:, 0:1], mv[:, 1:2]
                    y_sb = norm.tile([128, D], f32, tag="y")
                    if not apply_gamma_beta:
                        # combined LN+L2 row scale:
                        #   1/(sqrt(D*var) + eps_l2*sqrt(var+eps_ln))
                        a = stats_pool.tile([128, 1], f32, tag="a")
                        nc.scalar.activation(out=a, in_=var, func=Act.Sqrt,
                                             bias=zero_b, scale=float(D))
                        b2 = stats_pool.tile([128, 1], f32, tag="b2")
                        nc.scalar.activation(out=b2, in_=var, func=Act.Sqrt,
                                             bias=eps_b2, scale=L2_EPS * L2_EPS)
                        dn = stats_pool.tile([128, 1], f32, tag="dn")
                        nc.vector.tensor_tensor(out=dn, in0=a, in1=b2, op=Alu.add)
                        sc = stats_pool.tile([128, 1], f32, tag="sc")
                        nc.vector.reciprocal(out=sc, in_=dn)
                        nc.vector.tensor_scalar(out=y_sb, in0=p_ps, scalar1=mean,
                                                scalar2=sc, op0=Alu.subtract,
                                                op1=Alu.mult)
                    else:
                        std = stats_pool.tile([128, 1], f32, tag="std")
                        nc.scalar.activation(out=std, in_=var, func=Act.Sqrt,
                                             bias=eps_ln)
                        rstd = stats_pool.tile([128, 1], f32, tag="rstd")
                        nc.vector.reciprocal(out=rstd, in_=std)
                        nc.vector.tensor_scalar(out=y_sb, in0=p_ps, scalar1=mean,
                                                scalar2=rstd, op0=Alu.subtract,
                                                op1=Alu.mult)
                        nc.vector.tensor_tensor(out=y_sb, in0=y_sb,
                                                in1=gb_sb[g_name], op=Alu.mult)
                        nc.vector.tensor_tensor(out=y_sb, in0=y_sb,
                                                in1=gb_sb[b_name], op=Alu.add)
                        scr = norm.tile([128, D], f32, tag="scr")
                        ssq = stats_pool.tile([128, 1], f32, tag="ssq")
                        nc.scalar.activation(out=scr, in_=y_sb, func=Act.Square,
                                             accum_out=ssq)
                        dn = stats_pool.tile([128, 1], f32, tag="dn")
                        nc.scalar.activation(out=dn, in_=ssq, func=Act.Sqrt, bias=0.0)
                        dn2 = stats_pool.tile([128, 1], f32, tag="dn2")
                        nc.scalar.activation(out=dn2, in_=dn, func=Act.Copy,
                                             bias=L2_EPS, scale=1.0)
                        l2r = stats_pool.tile([128, 1], f32, tag="l2r")
                        nc.vector.reciprocal(out=l2r, in_=dn2)
                        nc.vector.tensor_scalar_mul(out=y_sb, in0=y_sb, scalar1=l2r)
                    return y_sb

                def store_T(y_sb, dst, col0):
                    """Transpose [128, D] -> dst[:, :, col0:col0+128] (f32r)."""
                    t_ps = ps_a.tile([128, 4, 128], f32, tag="ts")
                    for ci in range(4):
                        nc.tensor.transpose(
                            out=t_ps[:, ci, :],
                            in_=y_sb[:, ci * 128:(ci + 1) * 128], identity=ident)
                    nc.scalar.copy(out=dst[:, :, col0:col0 + 128], in_=t_ps)

                for t in range(n_kc):
                    p_ps = proj_tile(v_d, t * 128, wT["wv"])
                    nc.vector.tensor_copy(
                        out=vS[:, t, :, 0:DK],
                        in_=p_ps.rearrange("p (h d) -> p h d", h=H))

                for t in range(NQT):
                    p_ps = proj_tile(q_d, t * 128, wT["wq"])
                    y_sb = ln_l2(p_ps, "gq", "bq")
                    store_T(y_sb, qT, t * 128)

                for t in range(n_kc):
                    p_ps = proj_tile(k_d, t * 128, wT["wk"])
                    y_sb = ln_l2(p_ps, "gk", "bk")
                    store_T(y_sb, kT, t * 128)

                # ---- attention + output projection
                for qt in range(TOKQ // 512):
                    for h in range(8):
                        po = h % 2
                        ch = h // 2
                        o_ps = ps_o.tile([DK + 1, 512], f32, tag="o")
                        for kc in range(n_kc):
                            s_ps = ps_a.tile([128, 512], f32, tag="ts")
                            nc.tensor.matmul(
                                out=s_ps,
                                lhsT=kT[po * 64:(po + 1) * 64, ch,
                                        kc * 128:(kc + 1) * 128],
                                rhs=qT[po * 64:(po + 1) * 64, ch,
                                       qt * 512:(qt + 1) * 512],
                                start=True, stop=True)
                            pr_sb = probs_pool.tile([128, 512], f32r, tag="pr")
                            nc.scalar.activation(
                                out=pr_sb, in_=s_ps, func=Act.Exp,
                                bias=mask_bias[:, kc:kc + 1], scale=1.0 / 8.0)
                            nc.tensor.matmul(
                                out=o_ps, lhsT=vS[:, kc, h, :], rhs=pr_sb,
                                start=(kc == 0), stop=(kc == n_kc - 1))
                        recr_sb = stats_pool.tile([1, 512], f32r, tag="recr")
                        with nc.allow_low_precision(reason="f32r recip for bcast"):
                            nc.vector.reciprocal(out=recr_sb,
                                                 in_=o_ps[DK:DK + 1, :])
                        b_ps = ps_b.tile([64, 512], f32, tag="b")
                        nc.tensor.matmul(out=b_ps, lhsT=ones64, rhs=recr_sb,
                                         start=True, stop=True)
                        b_sb = probs_pool.tile([64, 512], f32, tag="b_sb")
                        nc.vector.tensor_copy(out=b_sb, in_=b_ps)
                        nc.vector.tensor_tensor(
                            out=oT[po * 64:(po + 1) * 64, ch,
                                   qt * 512:(qt + 1) * 512],
                            in0=o_ps[0:DK, :], in1=b_sb, op=Alu.mult)

                    for ts in range(4):
                        col0 = qt * 512 + ts * 128
                        f_ps = ps_pf.tile([128, D], f32, tag="pf")
                        for ci in range(4):
                            nc.tensor.matmul(out=f_ps,
                                             lhsT=oT[:, ci, col0:col0 + 128],
                                             rhs=wT["wo"][:, ci, :],
                                             start=(ci == 0), stop=(ci == 3))
                        f_sb = fin_pool.tile([128, D], f32, tag="f_sb")
                        nc.vector.tensor_copy(out=f_sb, in_=f_ps)
                        nc.sync.dma_start(out=out_d[col0:col0 + 128, :], in_=f_sb)

    nc.compile()
    return nc


def _get_nc(apply_gamma_beta, n_kc, repeat=1):
    key = (bool(apply_gamma_beta), int(n_kc), int(repeat))
    if key not in _cache:
        _cache[key] = _build(*key)
    return _cache[key]


def _prepare_in_maps(q, k, v, mask, Wq, Wk, Wv, Wo, gb_arrays=None):
    """Slice + compact inputs per core. Returns (in_maps, n_kc)."""
    idxs = [np.flatnonzero(mask[b_, 0, 0]) for b_ in range(B)]
    n_max = max(len(ix) for ix in idxs)
    if n_max == 0:
        n_max = 1
    n_kc = (n_max + 127) // 128
    SK = n_kc * 128

    in_maps = []
    for c in range(8):
        b_, h_ = c // 2, c % 2
        ix = idxs[b_]
        kc = np.zeros((SK, D), np.float32)
        vc = np.zeros((SK, D), np.float32)
        mc = np.zeros((SK,), np.int32)
        kc[:len(ix)] = k[b_][ix]
        vc[:len(ix)] = v[b_][ix]
        mc[:len(ix)] = 1
        m = {
            "q": np.ascontiguousarray(q[b_, h_ * TOKQ:(h_ + 1) * TOKQ]),
            "k": kc, "v": vc, "mask": mc,
            "wq": Wq, "wk": Wk, "wv": Wv, "wo": Wo,
        }
        if gb_arrays is not None:
            m.update(gb_arrays)
        in_maps.append(m)
    return in_maps, n_kc


def kernel(q, k, v, mask, Wq, Wk, Wv, Wo, gq, bq, gk, bk):
    from concourse.bass_utils import run_bass_kernel_spmd

    q = np.asarray(q, dtype=np.float32)
    k = np.asarray(k, dtype=np.float32)
    v = np.asarray(v, dtype=np.float32)
    mask = np.asarray(mask, dtype=np.int32)
    Wq, Wk = np.asarray(Wq, np.float32), np.asarray(Wk, np.float32)
    Wv, Wo = np.asarray(Wv, np.float32), np.asarray(Wo, np.float32)
    gq, bq = np.asarray(gq, np.float32), np.asarray(bq, np.float32)
    gk, bk = np.asarray(gk, np.float32), np.asarray(bk, np.float32)

    gb = not (np.all(gq == 1.0) and np.all(bq == 0.0)
              and np.all(gk == 1.0) and np.all(bk == 0.0))
    gb_arrays = {"gq": gq, "bq": bq, "gk": gk, "bk": bk} if gb else None
    in_maps, n_kc = _prepare_in_maps(q, k, v, mask, Wq, Wk, Wv, Wo, gb_arrays)
    nc = _get_nc(gb, n_kc)

    res = run_bass_kernel_spmd(nc, in_maps, core_ids=list(range(8)))
    out = np.empty((B, S, D), np.float32)
    for c in range(8):
        b_, h_ = c // 2, c % 2
        out[b_, h_ * TOKQ:(h_ + 1) * TOKQ] = res.results[c]["out"]
    return out
